# revision 3
# baseline (speedup 1.0000x reference)
"""Trainium2 Bass kernel for nn_Attention_41704132444382 (v4).

Masked-linear QKV + 16-head attention + masked-linear output projection,
tensor-parallel over heads across 8 NeuronCores (2 heads/core).

v4 structure (vs the 265us v3):
  - Softmax normalization moved ON DEVICE and fused into the PV-psum
    evacuation: the per-head V tiles carry 64 ones-columns (not 1), so the
    PV matmul replicates the denominator across psum rows 64..127 at zero
    PE cost (PE time scales with output free size only). A DVE reciprocal
    of rows 64:128 + one tensor_tensor mult per head normalizes during
    evacuation. No den outputs, no host division.
  - With at pre-normalized, the out-projection contracts over all 128
    head-dims (both heads) in ONE K=128 matmul per (token-group, oh):
    half of v3's out-proj PE work, and the two heads are summed in PSUM
    for free -> po output halves to [T, DIM].
  - fp16 throughout where range allows: x/wqkv (QKV inputs), at/wo
    (out-proj inputs), po output. exp values span 1e+-26 so e/v stay
    bf16. fp16 halves the x DMA (8MB) which was the phase-1 floor.
  - exp split between ScalarE (exact ACTIVATE) and DVE (Schraudolph
    int16(A*s+B) viewed as bf16): BASS_ATTN_SCHRAUD_JT j-tiles per block
    (default 8) go to DVE. po evacuations split DVE/Scalar via
    BASS_ATTN_PO_SCALAR (default 2 of 4 per block).

PSUM: qkv 4x[128,512] (phase 1) -> scores 2x[128,1024] (4 banks) +
pv 4x[128,512] (4, two per block double-buffered) ... actually
2 pools x 2 bufs: pv 2x[128,512]x2 = 4 banks? scores 4 + pv 2x1x2... see
pools: sps bufs=2 (4 banks) + pvps bufs=4 (4 banks of [128,512]) + po
shares via pops bufs=... (2 banks) -- total 8 when po overlaps next
block's scores; pvps bufs kept at 2-per-block with reuse.
"""

import math
import os
import sys

import numpy as np

sys.path.insert(0, "/opt/trn_rl_repo")

import concourse.bass as bass
import concourse.mybir as mybir
from concourse import bacc
from concourse.tile import TileContext

DIM = 1024
HEADS = 16
B = 2
N = 2048
T = B * N  # 4096 flattened tokens
NCORES = 8
DV = 128  # head-dims per core (2 heads x 64)
SCALE = DIM ** (-0.5)  # 1/32

F32 = mybir.dt.float32
F32R = mybir.dt.float32r
F16 = mybir.dt.float16
BF16 = mybir.dt.bfloat16
I16 = mybir.dt.int16

# number of j-tiles (of 16) per block whose exp runs on DVE (Schraudolph)
N_SCHRAUD = int(os.environ.get("BASS_ATTN_SCHRAUD_JT", "8"))
# of the 4 po evacuations per block, how many run on ScalarE (rest DVE)
PO_SCALAR = int(os.environ.get("BASS_ATTN_PO_SCALAR", "2"))
# bf16-bits variant: int16(A16*s + B16) viewed as bf16
SCHRAUD_A = (2.0 ** 7) / math.log(2.0) * SCALE
SCHRAUD_B = 127.0 * 128.0 - float(os.environ.get("BASS_ATTN_SCHRAUD_SIGMA", "7.42"))


def build_nc():
    nc = bacc.Bacc("TRN2", target_bir_lowering=True)
    xT_d = nc.declare_dram_parameter("xT", [DIM, T], F16, isOutput=False)
    wqkvT_d = nc.declare_dram_parameter("wqkvT", [DIM, 384], F16, isOutput=False)
    woT_d = nc.declare_dram_parameter("woT", [DV, DIM], F16, isOutput=False)
    po_d = nc.declare_dram_parameter("po", [T, DIM], F16, isOutput=True)

    mult = mybir.AluOpType.mult
    add = mybir.AluOpType.add
    Exp = mybir.ActivationFunctionType.Exp
    Copy = mybir.ActivationFunctionType.Copy

    # spread the DVE-exp tiles across the block (odd tiles first, then even)
    order = [15, 13, 11, 9, 7, 5, 3, 1, 14, 12, 10, 8, 6, 4, 2, 0]
    schraud_jt = set(order[: min(N_SCHRAUD, 16)])

    with TileContext(nc) as tc:
        with tc.tile_pool(name="persist", bufs=1) as pp:
            wqkv_g = pp.tile([128, 8 * 384], F16)  # [k-part, (kt, o)]
            wo_g = pp.tile([128, 1024], F16)
            qT = pp.tile([128, 4096], F32R)
            kTt = pp.tile([128, 4096], F32R)
            vT = pp.tile([128, 4096], BF16)
            # per j-tile blocks of 128: cols 0..63 = V, cols 64..127 = ones
            v1 = pp.tile([128, 32 * 128], BF16)  # head 0
            v2 = pp.tile([128, 32 * 128], BF16)  # head 1

            for vv in (v1, v2):
                nc.vector.memset(
                    vv[:].rearrange("p (j c) -> p j c", c=128)[:, :, 64:128], 1.0
                )

            # weight loads on the scalar queue
            nc.scalar.dma_start(
                wqkv_g[:].rearrange("p (kt o) -> p kt o", kt=8),
                wqkvT_d[:].rearrange("(kt p) o -> p kt o", p=128),
            )
            nc.scalar.dma_start(wo_g[:], woT_d[:])

            # ---------- Phase 1: QKV projection ----------
            with (
                tc.tile_pool(name="xq", bufs=16) as xp,
                tc.tile_pool(name="qk_ps", bufs=4, space="PSUM") as qkps,
            ):
                xq_tiles = {}

                def load_quarter(q, eng):
                    xq_tiles[q] = [
                        xp.tile([128, 1024], F16, tag="xq", name=f"xq{q}_{i}")
                        for i in range(8)
                    ]
                    for kt in range(8):
                        eng.dma_start(
                            xq_tiles[q][kt][:],
                            xT_d[kt * 128 : (kt + 1) * 128, q * 1024 : (q + 1) * 1024],
                        )

                DESTS = (qT, kTt, vT)

                def emit_chain(q, ot, th):
                    ps = qkps.tile([128, 512], F32, tag="qkps", name=f"qk{q}_{ot}_{th}")
                    for kt in range(8):
                        nc.tensor.matmul(
                            ps[:],
                            wqkv_g[:, kt * 384 + ot * 128 : kt * 384 + (ot + 1) * 128],
                            xq_tiles[q][kt][:, th * 512 : (th + 1) * 512],
                            start=(kt == 0),
                            stop=(kt == 7),
                        )
                    col = q * 1024 + th * 512
                    nc.vector.tensor_copy(DESTS[ot][:, col : col + 512], ps[:])

                load_quarter(0, nc.gpsimd)
                load_quarter(1, nc.gpsimd)
                load_quarter(2, nc.gpsimd)
                load_quarter(3, nc.gpsimd)
                for q in range(4):
                    for ot in range(3):
                        for th in range(2):
                            emit_chain(q, ot, th)

            # ---------- Phase 2: attention ----------
            with (
                tc.tile_pool(name="es", bufs=6) as ep,
                tc.tile_pool(name="at", bufs=2) as atp,
                tc.tile_pool(name="rc", bufs=2) as rcp,
                tc.tile_pool(name="ob", bufs=3) as obp,
                tc.tile_pool(name="vstg", bufs=2) as vsp,
                tc.tile_pool(name="s_ps", bufs=2, space="PSUM") as sps,
                tc.tile_pool(name="pv_ps", bufs=2, space="PSUM") as pvps,
                tc.tile_pool(name="po_ps", bufs=1, space="PSUM") as pops,
            ):
                def emit_vtransp(b):
                    # V^T [dv, t] -> V [t, dv] via DMA crossbar; contiguous
                    # staging then strided copy into the (dv|ones) layout
                    for h, vv in enumerate((v1, v2)):
                        vstg = vsp.tile([128, 1024], BF16, tag="vstg")
                        nc.sync.dma_start_transpose(
                            vstg[:].rearrange("p (j c) -> p j c", c=64),
                            vT[h * 64 : (h + 1) * 64, b * 2048 : (b + 1) * 2048],
                        )
                        nc.vector.tensor_copy(
                            vv[:].rearrange("p (j c) -> p j c", c=128)[
                                :, b * 16 : (b + 1) * 16, 0:64
                            ],
                            vstg[:].rearrange("p (j c) -> p j c", c=64),
                        )

                emit_vtransp(0)
                emit_vtransp(1)

                def emit_po(prev, tg):
                    # one K=128 matmul per oh-half: both heads contract and
                    # sum in psum (at is pre-normalized)
                    pb, at = prev
                    row = pb * 512 + tg * 128
                    po = pops.tile([128, 1024], F32, tag="po", name=f"po{pb}_{tg}")
                    for oh in range(2):
                        nc.tensor.matmul(
                            po[:, oh * 512 : (oh + 1) * 512],
                            at[:, tg * 128 : (tg + 1) * 128],
                            wo_g[:, oh * 512 : (oh + 1) * 512],
                            start=True,
                            stop=True,
                        )
                    ob = obp.tile([128, 1024], F16, tag="ob", name=f"ob{pb}_{tg}")
                    if tg < PO_SCALAR:
                        nc.scalar.activation(ob[:], po[:], Copy)
                    else:
                        nc.vector.tensor_copy(ob[:], po[:])
                    nc.sync.dma_start(po_d[row : row + 128, :], ob[:])

                # ---- attention blocks ----
                prev = None
                for bb in range(8):
                    b, ic = bb // 4, bb % 4
                    i0 = b * 2048 + ic * 512
                    pvs = [
                        pvps.tile([128, 512], F32, tag="pv", name=f"pv{bb}_{h}")
                        for h in range(2)
                    ]
                    for jt in range(16):
                        j0 = b * 2048 + jt * 128
                        jv = (b * 16 + jt) * 128
                        sg = sps.tile([128, 1024], F32, tag="s", name=f"s{bb}_{jt}")
                        for h in range(2):
                            nc.tensor.matmul(
                                sg[:, h * 512 : (h + 1) * 512],
                                kTt[h * 64 : (h + 1) * 64, j0 : j0 + 128],
                                qT[h * 64 : (h + 1) * 64, i0 : i0 + 512],
                                start=True,
                                stop=True,
                                tile_position=(h * 64, 0),
                            )
                        et = ep.tile([128, 1024], BF16, tag="e", name=f"e{bb}_{jt}")
                        if jt in schraud_jt:
                            nc.vector.tensor_scalar(
                                et[:].bitcast(I16),
                                sg[:],
                                SCHRAUD_A,
                                SCHRAUD_B,
                                mult,
                                add,
                            )
                        else:
                            nc.scalar.activation(et[:], sg[:], Exp, scale=SCALE)
                        for h, vv in enumerate((v1, v2)):
                            nc.tensor.matmul(
                                pvs[h][:],
                                vv[:, jv : jv + 128],
                                et[:, h * 512 : (h + 1) * 512],
                                start=(jt == 0),
                                stop=(jt == 15),
                            )
                        if prev is not None and jt in (1, 3, 5, 7):
                            emit_po(prev, (jt - 1) // 2)
                    # --- evacuate block: normalized attn^T via den rows ---
                    at = atp.tile([128, 512], F16, tag="at", name=f"at{bb}")
                    for h in range(2):
                        rec = rcp.tile([64, 512], F32, tag="rc", name=f"rc{bb}_{h}")
                        nc.vector.reciprocal(rec[:], pvs[h][64:128, :])
                        nc.vector.tensor_tensor(
                            at[h * 64 : (h + 1) * 64, :],
                            pvs[h][0:64, :],
                            rec[:],
                            mult,
                        )
                    prev = (bb, at)

                for tg in range(4):
                    emit_po(prev, tg)

    nc.compile()
    return nc


_NC = None


def _get_nc():
    global _NC
    if _NC is None:
        _NC = build_nc()
    return _NC


def _gate(mask):
    """Exact jax fp32 gate: sigmoid(m) > 0.5 (fp32 logistic rounding)."""
    mask = np.asarray(mask, dtype=np.float32)
    return (np.float32(1.0) / (np.float32(1.0) + np.exp(-mask))) > np.float32(0.5)


def make_in_maps(x, qkv_weight, qkv_weight_mask, out_weight, out_weight_mask):
    x = np.asarray(x, dtype=np.float32)
    wqkv = np.where(_gate(qkv_weight_mask), np.asarray(qkv_weight, np.float32), 0.0)
    wo = np.where(_gate(out_weight_mask), np.asarray(out_weight, np.float32), 0.0)

    xT = np.ascontiguousarray(x.reshape(T, DIM).T.astype(np.float16))
    in_maps = []
    for c in range(NCORES):
        r0 = c * DV
        sl = slice(r0, r0 + DV)
        w_shard = np.concatenate(
            [wqkv[sl], wqkv[DIM + r0 : DIM + r0 + DV], wqkv[2 * DIM + r0 : 2 * DIM + r0 + DV]],
            axis=0,
        )  # [384, 1024] rows = (q h0,h1 | k h0,h1 | v h0,h1)
        in_maps.append(
            {
                "xT": xT,
                "wqkvT": np.ascontiguousarray(w_shard.T.astype(np.float16)),
                "woT": np.ascontiguousarray(wo[:, sl].T.astype(np.float16)),
            }
        )
    return in_maps


LAST_RESULTS = None  # BassKernelResults of the most recent run (for profiling)


def kernel(
    x,
    qkv_weight,
    qkv_weight_mask,
    out_weight,
    out_weight_mask,
    out_bias,
    out_bias_mask,
    _trace=False,
    _tmpdir=None,
):
    global LAST_RESULTS
    from concourse.bass_utils import run_bass_kernel_spmd

    nc = _get_nc()
    in_maps = make_in_maps(x, qkv_weight, qkv_weight_mask, out_weight, out_weight_mask)
    res = run_bass_kernel_spmd(
        nc, in_maps, list(range(NCORES)), trace=_trace, tmpdir=_tmpdir
    )
    LAST_RESULTS = res
    out = np.zeros((T, DIM), dtype=np.float32)
    for r in res.results:
        out += np.asarray(r["po"]).astype(np.float32)
    out_bias = np.asarray(out_bias, dtype=np.float32)
    out += np.where(_gate(out_bias_mask), out_bias, 0.0)[None, :]
    return out.reshape(B, N, DIM)
